# revision 4
# baseline (speedup 1.0000x reference)
"""Trainium2 Bass kernel for nn_Attention: y = softmax((xW_q)(xW_k)^T/sqrt(d)) (xW_v).

Full inputs: x [16, 512, 4, 256] f32, W_qkv [768, 256] f32 (torch Linear layout).
The reference flattens (n, h) -> 2048 tokens and splits the 768 projection
outputs interleaved (stride 3) into q/k/v of width 256 each; attention runs
over the flat 2048-token axis with head dim 256.

Sharding: data-parallel over batch, 2 batches per core on 8 cores. W replicated.

Per-core device graph (all matmuls in float32r = full PE rate, fp32 storage):
  - x^T [256, 2048] per batch staged in SBUF; q^T, k^T = W^T-stationary matmuls
    ([o, i] layout); v = x^T-stationary matmuls ([j, o] layout) with a ones
    column appended (o=256) so P@V also yields the softmax row-sum.
  - S^T[j, i] tiles via k^T-stationary matmuls -> PSUM; ScalarE exp (scale
    fused, no max subtraction: |S*scale| <~ 6 for N(0,1) inputs) -> P^T in
    SBUF, already transposed for the P@V contraction over j.
  - P@V: P^T-stationary matmuls accumulate over the 16 j-chunks into
    [i-chunk, 257] PSUM; epilogue divides by the 257th column on VectorE.
Output [2, 2048, 256] per core; host concatenates and reshapes.
"""

import sys

for _p in ("/opt/trn_rl_repo",):
    if _p not in sys.path:
        sys.path.insert(0, _p)

import numpy as np

B, N, H, D = 16, 512, 4, 256
SEQ = N * H          # 2048 flat tokens
NCORES = 8
BPC = B // NCORES    # batches per core
SCALE = float(D) ** -0.5

_CACHE = {}


def _build_nc():
    import concourse.mybir as mybir
    import concourse.tile as tile
    from concourse import bacc

    f32 = mybir.dt.float32
    f32r = mybir.dt.float32r
    EXP = mybir.ActivationFunctionType.Exp

    nc = bacc.Bacc("TRN2", target_bir_lowering=False, debug=False)
    xT_ext = nc.declare_dram_parameter("xT", [BPC, D, SEQ], f32r, isOutput=False)
    wq_ext = nc.declare_dram_parameter("wq", [D, D], f32r, isOutput=False)
    wk_ext = nc.declare_dram_parameter("wk", [D, D], f32r, isOutput=False)
    wv_ext = nc.declare_dram_parameter("wv", [D, D], f32r, isOutput=False)
    out_ext = nc.declare_dram_parameter("out", [BPC, SEQ, D], f32, isOutput=True)

    DC = D // 128        # 2 contraction chunks of the 256-dim
    NJ = SEQ // 128      # 16 j-chunks
    NI = SEQ // 512      # 4 i-slices of 512
    VW = D + 2           # 258: v plus ones column (+pad: fp32r needs even N)

    with tile.TileContext(nc) as tc:
        with (
            tc.tile_pool(name="consts", bufs=1) as consts,
            tc.tile_pool(name="xt", bufs=2) as xt_pool,
            tc.tile_pool(name="qkv", bufs=2) as qkv_pool,
            tc.tile_pool(name="pt", bufs=10) as pt_pool,
            tc.tile_pool(name="eout", bufs=4) as eout_pool,
            tc.tile_pool(name="ppsum", bufs=2, space="PSUM") as ppsum,
            tc.tile_pool(name="spsum", bufs=2, space="PSUM") as spsum,
            tc.tile_pool(name="opsum", bufs=2, space="PSUM") as opsum,
        ):
            # Weights, [d, o] layout chunked on the 128-partition d axis.
            w_sb = {}
            for name, ext in (("q", wq_ext), ("k", wk_ext), ("v", wv_ext)):
                w = consts.tile([128, DC, D], f32r, tag=f"w{name}")
                for dc in range(DC):
                    nc.sync.dma_start(
                        out=w[:, dc, :], in_=ext[dc * 128 : (dc + 1) * 128, :]
                    )
                w_sb[name] = w
            ones_sb = consts.tile([128, 1], f32, tag="ones")
            nc.vector.memset(ones_sb[:], 1.0)

            for bb in range(BPC):
                xt = xt_pool.tile([128, DC, SEQ], f32r)
                for dc in range(DC):
                    nc.sync.dma_start(
                        out=xt[:, dc, :], in_=xT_ext[bb, dc * 128 : (dc + 1) * 128, :]
                    )

                # q^T and k^T: [o, i] layout, o on partitions in 2 chunks.
                qT = qkv_pool.tile([128, DC, SEQ], f32r, tag="qT")
                kT = qkv_pool.tile([128, DC, SEQ], f32r, tag="kT")
                for name, dst in (("q", qT), ("k", kT)):
                    for oc in range(DC):
                        for isl in range(NI):
                            ps = ppsum.tile([128, 512], f32)
                            for dc in range(DC):
                                nc.tensor.matmul(
                                    ps[:],
                                    w_sb[name][:, dc, oc * 128 : (oc + 1) * 128],
                                    xt[:, dc, isl * 512 : (isl + 1) * 512],
                                    start=(dc == 0),
                                    stop=(dc == DC - 1),
                                )
                            nc.vector.tensor_copy(
                                dst[:, oc, isl * 512 : (isl + 1) * 512], ps[:]
                            )

                # v: [j, o] layout + ones column for the softmax denominator.
                v_sb = qkv_pool.tile([128, NJ, VW], f32r, tag="v")
                nc.vector.tensor_copy(
                    v_sb[:, :, D:VW], ones_sb[:].to_broadcast([128, NJ, VW - D])
                )
                for jc in range(NJ):
                    ps = ppsum.tile([128, 512], f32)
                    for dc in range(DC):
                        nc.tensor.matmul(
                            ps[:, 0:D],
                            xt[:, dc, jc * 128 : (jc + 1) * 128],
                            w_sb["v"][:, dc, :],
                            start=(dc == 0),
                            stop=(dc == DC - 1),
                        )
                    nc.vector.tensor_copy(v_sb[:, jc, 0:D], ps[:, 0:D])

                # Attention, one 512-wide i-slice at a time.
                for isl in range(NI):
                    pts = []
                    for jp in range(NJ // 2):  # two j-chunks per PSUM tensor
                        sp = spsum.tile([128, 2, 512], f32)
                        for half in range(2):
                            jc = jp * 2 + half
                            for oc in range(DC):
                                nc.tensor.matmul(
                                    sp[:, half, :],
                                    kT[:, oc, jc * 128 : (jc + 1) * 128],
                                    qT[:, oc, isl * 512 : (isl + 1) * 512],
                                    start=(oc == 0),
                                    stop=(oc == DC - 1),
                                )
                        pt = pt_pool.tile([128, 2, 512], f32r)
                        nc.scalar.activation(pt[:], sp[:], EXP, scale=SCALE)
                        pts.append(pt)

                    for ic in range(4):
                        i0 = isl * 512 + ic * 128
                        op = opsum.tile([128, VW], f32)
                        for jc in range(NJ):
                            nc.tensor.matmul(
                                op[:],
                                pts[jc // 2][:, jc % 2, ic * 128 : (ic + 1) * 128],
                                v_sb[:, jc, :],
                                start=(jc == 0),
                                stop=(jc == NJ - 1),
                            )
                        rec = eout_pool.tile([128, 1], f32, tag="rec")
                        nc.vector.reciprocal(rec[:], op[:, D : D + 1])
                        osb = eout_pool.tile([128, D], f32, tag="osb")
                        nc.vector.tensor_scalar_mul(osb[:], op[:, 0:D], rec[:])
                        nc.sync.dma_start(
                            out=out_ext[bb, i0 : i0 + 128, :], in_=osb[:]
                        )

    nc.compile()
    return nc


def _get_nc():
    if "nc" not in _CACHE:
        _CACHE["nc"] = _build_nc()
    return _CACHE["nc"]


def _prep_in_maps(x, W_qkv):
    x = np.ascontiguousarray(x, dtype=np.float32)
    W = np.ascontiguousarray(W_qkv, dtype=np.float32)
    xT = np.ascontiguousarray(x.reshape(B, SEQ, D).transpose(0, 2, 1))
    wqT = np.ascontiguousarray(W[0::3, :].T)
    wkT = np.ascontiguousarray(W[1::3, :].T)
    wvT = np.ascontiguousarray(W[2::3, :].T)
    return [
        {"xT": xT[c * BPC : (c + 1) * BPC], "wq": wqT, "wk": wkT, "wv": wvT}
        for c in range(NCORES)
    ]


def _run(x, W_qkv, trace=False, tmpdir=None):
    from concourse.bass_utils import run_bass_kernel_spmd

    nc = _get_nc()
    in_maps = _prep_in_maps(x, W_qkv)
    res = run_bass_kernel_spmd(
        nc, in_maps, core_ids=list(range(NCORES)), trace=trace, tmpdir=tmpdir
    )
    out = np.concatenate([res.results[c]["out"] for c in range(NCORES)], axis=0)
    return out.reshape(B, N, H, D).astype(np.float32), res


def kernel(x, W_qkv):
    out, _ = _run(x, W_qkv)
    return out


# revision 6
# speedup vs baseline: 1.2944x; 1.2944x over previous
"""Trainium2 Bass kernel for nn_Attention: y = softmax((xW_q)(xW_k)^T/sqrt(d)) (xW_v).

Full inputs: x [16, 512, 4, 256] f32, W_qkv [768, 256] f32 (torch Linear layout).
The reference flattens (n, h) -> 2048 tokens and splits the 768 projection
outputs interleaved (stride 3) into q/k/v of width 256 each; attention runs
over the flat 2048-token axis with head dim 256.

Sharding: data-parallel over batch, 2 batches per core on 8 cores. W replicated.

Per-core device graph:
  - x^T [256, 2048] per batch staged in SBUF (float32r = full PE rate, fp32
    storage); q^T, k^T = W^T-stationary fp32r matmuls ([o, i] layout); v =
    x^T-stationary matmuls, cast to bf16 in [j, o] layout with a ones column
    (so P@V also accumulates the softmax row-sum) plus one pad column.
  - S^T[j, i] tiles via k^T-stationary fp32r matmuls -> PSUM; ScalarE exp
    (scale fused; no max subtraction: |S*scale| <~ 6 for N(0,1) inputs)
    writes P^T to SBUF in bf16, already transposed for the P@V contraction.
  - P@V in bf16 (bf16 LDWEIGHTS hides under the N=258 matmul; fp32r would be
    weight-load-bound at ~2x the cost): P^T-stationary matmuls accumulate
    over the 16 j-chunks into [i-chunk, 258] f32 PSUM; epilogue multiplies
    by the reciprocal of the ones-column on VectorE.
  - P@V matmuls of slice s are emitted interleaved between the S^T groups of
    slice s+1 so the PE never stalls waiting on ScalarE's exp drain.
Output [2, 2048, 256] per core; host concatenates and reshapes.
"""

import sys

for _p in ("/opt/trn_rl_repo",):
    if _p not in sys.path:
        sys.path.insert(0, _p)

import numpy as np

B, N, H, D = 16, 512, 4, 256
SEQ = N * H          # 2048 flat tokens
NCORES = 8
BPC = B // NCORES    # batches per core
SCALE = float(D) ** -0.5

_CACHE = {}


def _build_nc():
    import concourse.mybir as mybir
    import concourse.tile as tile
    from concourse import bacc

    f32 = mybir.dt.float32
    f32r = mybir.dt.float32r
    bf16 = mybir.dt.bfloat16
    EXP = mybir.ActivationFunctionType.Exp

    nc = bacc.Bacc("TRN2", target_bir_lowering=False, debug=False)
    xT_ext = nc.declare_dram_parameter("xT", [BPC, D, SEQ], f32r, isOutput=False)
    wq_ext = nc.declare_dram_parameter("wq", [D, D], f32r, isOutput=False)
    wk_ext = nc.declare_dram_parameter("wk", [D, D], f32r, isOutput=False)
    wv_ext = nc.declare_dram_parameter("wv", [D, D], f32r, isOutput=False)
    out_ext = nc.declare_dram_parameter("out", [BPC, SEQ, D], f32, isOutput=True)

    DC = D // 128        # 2 contraction chunks of the 256-dim
    NJ = SEQ // 128      # 16 j-chunks
    NI = SEQ // 512      # 4 i-slices of 512
    VW = D + 2           # 258: v plus ones column plus pad

    with tile.TileContext(nc) as tc:
        with (
            tc.tile_pool(name="consts", bufs=1) as consts,
            tc.tile_pool(name="xt", bufs=2) as xt_pool,
            tc.tile_pool(name="qkv", bufs=2) as qkv_pool,
            tc.tile_pool(name="pt", bufs=20) as pt_pool,
            tc.tile_pool(name="eout", bufs=4) as eout_pool,
            tc.tile_pool(name="ps2", bufs=3, space="PSUM") as ps2,
            tc.tile_pool(name="ps1", bufs=2, space="PSUM") as ps1,
        ):
            w_sb = {}
            for name, ext in (("q", wq_ext), ("k", wk_ext), ("v", wv_ext)):
                w = consts.tile([128, DC, D], f32r, tag=f"w{name}")
                for dc in range(DC):
                    nc.sync.dma_start(
                        out=w[:, dc, :], in_=ext[dc * 128 : (dc + 1) * 128, :]
                    )
                w_sb[name] = w
            ones_sb = consts.tile([128, 1], f32, tag="ones")
            nc.vector.memset(ones_sb[:], 1.0)

            # One deferred P@V phase: emit_pav(prev, g) emits 1/8th of the
            # previous slice's P@V matmuls (one half of one i-chunk's 16-step
            # accumulation), so they slot between S^T groups on the PE.
            op_live = {}

            def emit_pav(prev, g):
                if prev is None:
                    return
                bb, isl, pts, v_prev = prev
                ic = g // 2
                half = g % 2
                if half == 0:
                    op_tile = ps1.tile([128, VW], f32, tag="ps1")
                    op_live[ic] = op_tile
                op = op_live[ic]
                for jc in range(half * 8, half * 8 + 8):
                    nc.tensor.matmul(
                        op[:],
                        pts[jc // 2][:, jc % 2, ic * 128 : (ic + 1) * 128],
                        v_prev[:, jc, :],
                        start=(jc == 0),
                        stop=(jc == NJ - 1),
                    )
                if half == 1:
                    rec = eout_pool.tile([128, 1], f32, tag="rec")
                    nc.vector.reciprocal(rec[:], op[:, D : D + 1])
                    osb = eout_pool.tile([128, D], f32, tag="osb")
                    nc.vector.tensor_scalar_mul(osb[:], op[:, 0:D], rec[:])
                    i0 = isl * 512 + ic * 128
                    nc.sync.dma_start(out=out_ext[bb, i0 : i0 + 128, :], in_=osb[:])
                    del op_live[ic]

            prev = None
            for bb in range(BPC):
                xt = xt_pool.tile([128, DC, SEQ], f32r)
                for dc in range(DC):
                    nc.sync.dma_start(
                        out=xt[:, dc, :], in_=xT_ext[bb, dc * 128 : (dc + 1) * 128, :]
                    )

                # q^T, k^T: [o, i] fp32r; v: [j, o]+ones in bf16.
                # The previous slice's P@V interleaves into the q/k groups.
                qT = qkv_pool.tile([128, DC, SEQ], f32r, tag="qT")
                kT = qkv_pool.tile([128, DC, SEQ], f32r, tag="kT")
                v_sb = qkv_pool.tile([128, NJ, VW], bf16, tag="v")
                nc.vector.tensor_copy(
                    v_sb[:, :, D:VW], ones_sb[:].to_broadcast([128, NJ, VW - D])
                )
                pg = 0
                for name, dst in (("q", qT), ("k", kT)):
                    for oc in range(DC):
                        for ih in range(2):
                            ps = ps2.tile([128, 2, 512], f32, tag="ps2")
                            for half in range(2):
                                isl = ih * 2 + half
                                for dc in range(DC):
                                    nc.tensor.matmul(
                                        ps[:, half, :],
                                        w_sb[name][:, dc, oc * 128 : (oc + 1) * 128],
                                        xt[:, dc, isl * 512 : (isl + 1) * 512],
                                        start=(dc == 0),
                                        stop=(dc == DC - 1),
                                    )
                            nc.vector.tensor_copy(
                                dst[:, oc, ih * 1024 : (ih + 1) * 1024],
                                ps[:].rearrange("p a b -> p (a b)"),
                            )
                            emit_pav(prev, pg)
                            pg += 1
                for jc in range(NJ):
                    ps = ps1.tile([128, D], f32, tag="ps1")
                    for dc in range(DC):
                        nc.tensor.matmul(
                            ps[:],
                            xt[:, dc, jc * 128 : (jc + 1) * 128],
                            w_sb["v"][:, dc, :],
                            start=(dc == 0),
                            stop=(dc == DC - 1),
                        )
                    nc.vector.tensor_copy(v_sb[:, jc, 0:D], ps[:])
                prev = None  # its P@V was flushed in the 8 q/k groups above

                for isl in range(NI):
                    pts = []
                    for g in range(8):  # two j-chunks per S^T group
                        sp = ps2.tile([128, 2, 512], f32, tag="ps2")
                        for half in range(2):
                            jc = g * 2 + half
                            for oc in range(DC):
                                nc.tensor.matmul(
                                    sp[:, half, :],
                                    kT[:, oc, jc * 128 : (jc + 1) * 128],
                                    qT[:, oc, isl * 512 : (isl + 1) * 512],
                                    start=(oc == 0),
                                    stop=(oc == DC - 1),
                                )
                        pt = pt_pool.tile([128, 2, 512], bf16)
                        nc.scalar.activation(pt[:], sp[:], EXP, scale=SCALE)
                        pts.append(pt)
                        emit_pav(prev, g)
                    prev = (bb, isl, pts, v_sb)

            for g in range(8):  # flush the final slice's P@V
                emit_pav(prev, g)

    nc.compile()
    return nc


def _get_nc():
    if "nc" not in _CACHE:
        _CACHE["nc"] = _build_nc()
    return _CACHE["nc"]


def _prep_in_maps(x, W_qkv):
    x = np.ascontiguousarray(x, dtype=np.float32)
    W = np.ascontiguousarray(W_qkv, dtype=np.float32)
    xT = np.ascontiguousarray(x.reshape(B, SEQ, D).transpose(0, 2, 1))
    wqT = np.ascontiguousarray(W[0::3, :].T)
    wkT = np.ascontiguousarray(W[1::3, :].T)
    wvT = np.ascontiguousarray(W[2::3, :].T)
    return [
        {"xT": xT[c * BPC : (c + 1) * BPC], "wq": wqT, "wk": wkT, "wv": wvT}
        for c in range(NCORES)
    ]


def _run(x, W_qkv, trace=False, tmpdir=None):
    from concourse.bass_utils import run_bass_kernel_spmd

    nc = _get_nc()
    in_maps = _prep_in_maps(x, W_qkv)
    res = run_bass_kernel_spmd(
        nc, in_maps, core_ids=list(range(NCORES)), trace=trace, tmpdir=tmpdir
    )
    out = np.concatenate([res.results[c]["out"] for c in range(NCORES)], axis=0)
    return out.reshape(B, N, H, D).astype(np.float32), res


def kernel(x, W_qkv):
    out, _ = _run(x, W_qkv)
    return out
